# revision 3
# baseline (speedup 1.0000x reference)
"""Trainium2 Bass kernel for nn_Actions_block_14388140442036 (gnn_message_passing).

The reference network is entirely linear (no activations), so the output
    out = segment_sum(actions) @ pol_W + pol_b
collapses to per-effect scalars:
    p[j] = actions[j] @ pol_W  (a dot product against fused weight vectors)
followed by a scalar segment-sum.  Folding pol_W through each branch:

  glob branch:  p_g[i] = (globs @ w1)[U[i]]     + action_globs[i]. w2 + cg
  node branch:  p_n[i] = (nodes @ w3)[V[i]]     + action_nodes[i]. w4 + cn
  edge branch:  p_e[i] = (edges @ u1)[E[i]] + (nodes @ wr)[row[E[i]]]
                        + (nodes @ wc)[col[E[i]]] + action_edges[i]. wv + ce

where  w1|w2 = glob_W @ pol_W,  w3|w4 = node_W @ pol_W,
       u1|u2 = e2_W @ pol_W,    wr|wv|wc = e1_W @ u2.

The memory-heavy work -- streaming edges (205MB) and nodes (51MB, x3 weight
vectors) through per-row dot products -- runs on 8 NeuronCores (row-sharded,
replicated weight tiles).  Per 128-row group the PE transposes the tile
(fp32 has no DMA transpose), DVE/ACT alternate copying the transposed slab
back to SBUF, and the PE then matmuls it against the fused weight columns,
accumulating dot-product columns directly in PSUM banks.  The small action-
feature matvecs run on the DVE (mul + 3D-view reduce).  The host does the
tiny fused-weight precompute, the scalar gathers and the segment sum.
"""

import numpy as np

import concourse.bacc as bacc
import concourse.mybir as mybir
import concourse.tile as tile
from concourse.bass_utils import run_bass_kernel_spmd
from concourse.masks import make_identity

# ---- problem constants (hardcoded; kernel.py must be self-contained) ----
HID = 128
FEAT = 16
N_NODES = 100000
N_EDGES = 400000
N_PER = 100000
A_TOTAL = 300000
NUM_ACTIONS = 75000
N_CORES = 8

E_SH = N_EDGES // N_CORES   # 50000 edge rows per core
N_SH = N_NODES // N_CORES   # 12500 node rows per core
A_SH = N_PER // N_CORES     # 12500 action-effect rows per core

# Row->SBUF packing: C consecutive rows per partition, so a [128, C*128] tile
# covers 128*C rows with C*512B contiguous DMA descriptors.  C=4 so one tile
# is one PSUM bank worth of transposes.
C = 4
T_E = 97            # 97*512 = 49664 main edge rows
E_MED = 256         # rows 49664..49920 as [128, 256] (C=2)
E_TAIL = 80         # rows 49920..50000 as [80, 128]
T_N = 24            # 24*512 = 12288 main node rows
N_TAIL = 212        # rows 12288..12500 as [106, 256] (C=2)

C_A = 32            # apack: tile [128, 1536] covers 4096 rows (48 floats/row)
T_A = 3             # 3*4096 = 12288 main rows, tail 212 rows -> [106, 96]
A_TAIL = 212

QE_COLS = T_E * C + 3            # 391: 388 main + 2 med + 1 tail
QN_COLS = (T_N * C + 2) * 3      # 294: 98 groups x 3 weights
PA_COLS = T_A * C_A * 3 + 6      # 294

# wts input [128, 1540]: [0:3]=Wn columns (w3|wr|wc), [3:4]=u1 column,
# [4:1540]=w48 (=[w2|w4|wv]) tiled x32 replicated across partitions
W_N = (0, 3)
W_U1 = (3, 4)
W_A48 = (4, 4 + C_A * 48)
WTS_COLS = 4 + C_A * 48

F32 = mybir.dt.float32
AX = mybir.AxisListType.X

_CACHE = {}


def _build_program():
    nc = bacc.Bacc("TRN2", target_bir_lowering=False, debug=False,
                   num_devices=N_CORES)

    edges_in = nc.dram_tensor("edges_in", [E_SH, HID], F32, kind="ExternalInput").ap()
    nodes_in = nc.dram_tensor("nodes_in", [N_SH, HID], F32, kind="ExternalInput").ap()
    apack_in = nc.dram_tensor("apack_in", [A_SH, 3 * FEAT], F32, kind="ExternalInput").ap()
    wts_in = nc.dram_tensor("wts_in", [128, WTS_COLS], F32, kind="ExternalInput").ap()

    qe_out = nc.dram_tensor("qe_out", [128, QE_COLS], F32, kind="ExternalOutput").ap()
    qn_out = nc.dram_tensor("qn_out", [128, QN_COLS], F32, kind="ExternalOutput").ap()
    pa_out = nc.dram_tensor("pa_out", [128, PA_COLS], F32, kind="ExternalOutput").ap()

    with tile.TileContext(nc) as tc:
        with (
            tc.tile_pool(name="wpool", bufs=1) as wpool,
            tc.tile_pool(name="dpool", bufs=4) as dpool,
            tc.tile_pool(name="adpool", bufs=2) as adpool,
            tc.tile_pool(name="dtpool", bufs=4) as dtpool,
            tc.tile_pool(name="atpool", bufs=2) as atpool,
            tc.tile_pool(name="accpool", bufs=1) as accpool,
            tc.tile_pool(name="pstr", bufs=4, space="PSUM") as pstr,
            tc.tile_pool(name="psacc", bufs=1, space="PSUM") as psacc,
        ):
            wt = wpool.tile([128, WTS_COLS], F32)
            nc.sync.dma_start(wt[:], wts_in[:])
            ident = wpool.tile([128, 128], F32)
            make_identity(nc, ident[:])
            wn_col = wt[:, W_N[0]:W_N[1]]
            u1_col = wt[:, W_U1[0]:W_U1[1]]
            a48b = wt[:, W_A48[0]:W_A48[1]]

            qe_ps = psacc.tile([128, QE_COLS], F32)
            qn_ps = psacc.tile([128, QN_COLS], F32)
            pa_acc = accpool.tile([128, PA_COLS], F32)

            e_main = edges_in[0:T_E * 128 * C, :].rearrange(
                "(t p c) f -> t p (c f)", p=128, c=C)
            e_med = edges_in[T_E * 128 * C:T_E * 128 * C + E_MED, :].rearrange(
                "(p c) f -> p (c f)", c=2)
            n_main = nodes_in[0:T_N * 128 * C, :].rearrange(
                "(t p c) f -> t p (c f)", p=128, c=C)
            n_tail = nodes_in[T_N * 128 * C:N_SH, :].rearrange(
                "(p c) f -> p (c f)", c=2)

            # task list: (src_ap, parts, n_groups, rhs_ap, acc_ps, [out cols])
            tasks = []
            for t in range(T_E):
                tasks.append((e_main[t], 128, C, u1_col, qe_ps,
                              [(t * C + g, 1) for g in range(C)]))
            tasks.append((e_med, 128, 2, u1_col, qe_ps,
                          [(T_E * C + g, 1) for g in range(2)]))
            tasks.append((edges_in[T_E * 128 * C + E_MED:E_SH, :], E_TAIL, 1,
                          u1_col, qe_ps, [(T_E * C + 2, 1)]))
            for t in range(T_N):
                tasks.append((n_main[t], 128, C, wn_col, qn_ps,
                              [((t * C + g) * 3, 3) for g in range(C)]))
            tasks.append((n_tail, 106, 2, wn_col, qn_ps,
                          [((T_N * C + g) * 3, 3) for g in range(2)]))

            # 2-stage software pipeline: emit dots two tiles behind the
            # transposes so PE never stalls on the PSUM->SBUF copy.
            pending = []

            def emit_dots():
                parts, n_groups, rhs, acc, cols, dT = pending.pop(0)
                for g in range(n_groups):
                    c0, ncol = cols[g]
                    nc.tensor.matmul(
                        acc[:parts, c0:c0 + ncol],
                        dT[:, g * 128:g * 128 + parts],
                        rhs[:, :])

            for i, (src, parts, n_groups, rhs, acc, cols) in enumerate(tasks):
                d = dpool.tile([128, C * HID], F32, tag="d")
                nc.sync.dma_start(d[:parts, :n_groups * HID], src)
                ps = pstr.tile([128, C * HID], F32, tag="ps")
                for g in range(n_groups):
                    nc.tensor.transpose(
                        ps[:, g * 128:g * 128 + parts],
                        d[:parts, g * 128:(g + 1) * 128],
                        ident[:parts, :parts])
                dT = dtpool.tile([128, C * HID], F32, tag="dT")
                w = n_groups * 128 if parts == 128 else n_groups * 128  # slab width
                if i % 2 == 0:
                    nc.vector.tensor_copy(dT[:, :w], ps[:, :w])
                else:
                    nc.scalar.copy(dT[:, :w], ps[:, :w])
                pending.append((parts, n_groups, rhs, acc, cols, dT))
                if len(pending) > 2:
                    emit_dots()
            while pending:
                emit_dots()

            # ---- action features on DVE: pa = [ag|an|ae] . [w2|w4|wv] ----
            a_main = apack_in[0:T_A * 128 * C_A, :].rearrange(
                "(t p c) f -> t p (c f)", p=128, c=C_A)
            for t in range(T_A):
                d = adpool.tile([128, C_A * 48], F32, tag="ad")
                nc.sync.dma_start(d[:], a_main[t])
                tmp = atpool.tile([128, C_A * 48], F32, tag="at")
                nc.vector.tensor_mul(tmp[:], d[:], a48b)
                nc.vector.reduce_sum(
                    pa_acc[:, t * C_A * 3:(t + 1) * C_A * 3],
                    tmp[:].rearrange("p (s f) -> p s f", f=FEAT), axis=AX)
            a_tail = apack_in[T_A * 128 * C_A:A_SH, :].rearrange(
                "(p c) f -> p (c f)", c=2)
            AP_T = A_TAIL // 2  # 106
            d = adpool.tile([128, 96], F32, tag="ad")
            nc.sync.dma_start(d[:AP_T, :], a_tail)
            tmp = atpool.tile([128, 96], F32, tag="at")
            nc.vector.tensor_mul(tmp[:AP_T, :], d[:AP_T, :], a48b[:AP_T, :96])
            nc.vector.reduce_sum(
                pa_acc[:AP_T, T_A * C_A * 3:T_A * C_A * 3 + 6],
                tmp[:AP_T, :].rearrange("p (s f) -> p s f", f=FEAT), axis=AX)

            # ---- drain accumulators ----
            qe_sb = accpool.tile([128, QE_COLS], F32)
            qn_sb = accpool.tile([128, QN_COLS], F32)
            nc.vector.tensor_copy(qe_sb[:], qe_ps[:])
            nc.vector.tensor_copy(qn_sb[:], qn_ps[:])
            nc.sync.dma_start(qe_out[:], qe_sb[:])
            nc.sync.dma_start(qn_out[:], qn_sb[:])
            nc.sync.dma_start(pa_out[:], pa_acc[:])

    nc.compile()
    return nc


def _get_program():
    if "nc" not in _CACHE:
        _CACHE["nc"] = _build_program()
    return _CACHE["nc"]


def _unscramble_qe(arr):
    """[128, 391] -> [50000] in original row order."""
    main = arr[:, :T_E * C].reshape(128, T_E, C).transpose(1, 0, 2).reshape(-1)
    med = arr[:, T_E * C:T_E * C + 2].reshape(-1)          # rows 49664 + p*2+g
    tail = arr[:E_TAIL, T_E * C + 2]                       # rows 49920 + p
    return np.concatenate([main, med, tail])


def _unscramble_qn(arr):
    """[128, 294] -> [12500, 3] (w3, wr, wc dots) in original row order."""
    main = arr[:, :T_N * C * 3].reshape(128, T_N, C, 3).transpose(1, 0, 2, 3)
    main = main.reshape(-1, 3)                             # rows t*512+p*4+g
    tail = arr[:106, T_N * C * 3:].reshape(106, 2, 3).reshape(-1, 3)
    return np.concatenate([main, tail], axis=0)


def _unscramble_pa(arr):
    """[128, 294] -> [12500, 3] (ag.w2, an.w4, ae.wv) in original row order."""
    main = arr[:, :T_A * C_A * 3].reshape(128, T_A, C_A, 3).transpose(1, 0, 2, 3)
    main = main.reshape(-1, 3)
    tail = arr[:A_TAIL // 2, T_A * C_A * 3:].reshape(-1, 3)
    return np.concatenate([main, tail], axis=0)


def kernel(**inputs):
    globs = inputs["globs"]
    nodes = np.ascontiguousarray(inputs["nodes"])
    edges = np.ascontiguousarray(inputs["edges"])
    action_globs = inputs["action_globs"]
    action_nodes = inputs["action_nodes"]
    action_edges = inputs["action_edges"]
    glob_W = inputs["glob_W"]; glob_b = inputs["glob_b"]
    node_W = inputs["node_W"]; node_b = inputs["node_b"]
    e1_W = inputs["e1_W"]; e1_b = inputs["e1_b"]
    e2_W = inputs["e2_W"]; e2_b = inputs["e2_b"]
    pol_W = inputs["pol_W"]; pol_b = inputs["pol_b"]
    row = inputs["row"]; col = inputs["col"]
    U = inputs["U"]; UA = inputs["UA"]; V = inputs["V"]; VA = inputs["VA"]
    E = inputs["E"]; EA = inputs["EA"]
    actions_batch = inputs["actions_batch"]

    # ---- fused weight vectors (float64 for accuracy; cast to f32 on device) ----
    polW = pol_W.astype(np.float64)[:, 0]                 # [128]
    g_f = glob_W.astype(np.float64) @ polW                # [144]
    n_f = node_W.astype(np.float64) @ polW                # [144]
    e2_f = e2_W.astype(np.float64) @ polW                 # [256]
    u1, u2 = e2_f[:HID], e2_f[HID:]
    e1_f = e1_W.astype(np.float64) @ u2                   # [272]
    w1, w2 = g_f[:HID], g_f[HID:]
    w3, w4 = n_f[:HID], n_f[HID:]
    wr, wv, wc = e1_f[:HID], e1_f[HID:HID + FEAT], e1_f[HID + FEAT:]
    cg = float(glob_b.astype(np.float64) @ polW)
    cn = float(node_b.astype(np.float64) @ polW)
    ce = float(e2_b.astype(np.float64) @ polW + e1_b.astype(np.float64) @ u2)

    wts = np.zeros((128, WTS_COLS), np.float32)
    wts[:, W_N[0]] = w3.astype(np.float32)
    wts[:, W_N[0] + 1] = wr.astype(np.float32)
    wts[:, W_N[0] + 2] = wc.astype(np.float32)
    wts[:, W_U1[0]] = u1.astype(np.float32)
    w48 = np.concatenate([w2, w4, wv]).astype(np.float32)
    wts[:, W_A48[0]:W_A48[1]] = np.tile(w48, (128, C_A))

    # packed action features [N_PER, 48] = [ag | an | ae]
    apack = np.empty((N_PER, 3 * FEAT), np.float32)
    apack[:, :FEAT] = action_globs
    apack[:, FEAT:2 * FEAT] = action_nodes
    apack[:, 2 * FEAT:] = action_edges

    nc = _get_program()
    in_maps = []
    for c in range(N_CORES):
        in_maps.append({
            "edges_in": edges[c * E_SH:(c + 1) * E_SH],
            "nodes_in": nodes[c * N_SH:(c + 1) * N_SH],
            "apack_in": apack[c * A_SH:(c + 1) * A_SH],
            "wts_in": wts,
        })
    res = run_bass_kernel_spmd(nc, in_maps, core_ids=list(range(N_CORES)))

    qe = np.empty(N_EDGES, np.float64)
    qn3 = np.empty((N_NODES, 3), np.float64)
    pa = np.empty((N_PER, 3), np.float64)
    for c in range(N_CORES):
        r = res.results[c]
        qe[c * E_SH:(c + 1) * E_SH] = _unscramble_qe(r["qe_out"])
        qn3[c * N_SH:(c + 1) * N_SH] = _unscramble_qn(r["qn_out"])
        pa[c * A_SH:(c + 1) * A_SH] = _unscramble_pa(r["pa_out"])
    qn, qr, qc = qn3[:, 0], qn3[:, 1], qn3[:, 2]

    # ---- host: gathers, scatter into action slots, segment sum ----
    qg = globs.astype(np.float64) @ w1                    # [512]
    p_g = qg[U] + pa[:, 0] + cg
    p_n = qn[V] + pa[:, 1] + cn
    p_e = qe[E] + qr[row[E]] + qc[col[E]] + pa[:, 2] + ce

    actions_p = np.zeros(A_TOTAL, np.float64)
    actions_p[UA] = p_g
    actions_p[VA] = p_n
    actions_p[EA] = p_e

    # torch-style _norm: consecutive group ids starting at actions_batch[0]
    ab = actions_batch.astype(np.int64)
    changed = ab[1:] != ab[:-1]
    seg = int(ab[0]) + np.concatenate([[0], np.cumsum(changed)])
    if seg[0] >= 0 and seg[-1] < NUM_ACTIONS:
        agg = np.bincount(seg, weights=actions_p, minlength=NUM_ACTIONS)[:NUM_ACTIONS]
    else:  # jax segment_sum drops out-of-range ids
        agg = np.zeros(NUM_ACTIONS, np.float64)
        valid = (seg >= 0) & (seg < NUM_ACTIONS)
        np.add.at(agg, seg[valid], actions_p[valid])

    out = agg + float(pol_b.astype(np.float64)[0])
    return out.astype(np.float32)[:, None]


# revision 9
# speedup vs baseline: 2.3555x; 2.3555x over previous
"""Trainium2 Bass kernel for nn_Actions_block_14388140442036 (gnn_message_passing).

The reference network is entirely linear (no activations), so the output
    out = segment_sum(actions) @ pol_W + pol_b
collapses to per-effect scalars:
    p[j] = actions[j] @ pol_W  (a dot product against fused weight vectors)
followed by a scalar segment-sum.  Folding pol_W through each branch:

  glob branch:  p_g[i] = (globs @ w1)[U[i]]     + action_globs[i]. w2 + cg
  node branch:  p_n[i] = (nodes @ w3)[V[i]]     + action_nodes[i]. w4 + cn
  edge branch:  p_e[i] = (edges @ u1)[E[i]] + (nodes @ wr)[row[E[i]]]
                        + (nodes @ wc)[col[E[i]]] + action_edges[i]. wv + ce

where  w1|w2 = glob_W @ pol_W,  w3|w4 = node_W @ pol_W,
       u1|u2 = e2_W @ pol_W,    wr|wv|wc = e1_W @ u2.

The memory-heavy work -- streaming edges (205MB), nodes (51MB, x3 weight
vectors) and packed action features (19MB) through per-row dot products --
runs on 8 NeuronCores (row-sharded, replicated weight columns).  Large C=16
row-packed DMA tiles (8KB descriptors, few dma_starts: the HWDGE trigger is
~625ns serialized per DMA; small remainder tiles are issued first so their
trigger latency hides under the pipeline ramp).  Per 128-row group the PE
transposes the tile (fp32 has no DMA transpose), DVE/ACT alternate copying
4-group PSUM slabs back to SBUF, and the PE then matmuls them against the
fused weight columns, accumulating dot-product columns directly in PSUM
banks.  The small action-feature matvecs run on the DVE (mul + 3D-view
reduce) in small chunks interleaved with the slab copies.  Each branch's
accumulator drains to HBM as soon as its last dots are emitted.  The host
does the tiny fused-weight precompute, the scalar gathers and the segment
sum.
"""

import numpy as np

import concourse.bacc as bacc
import concourse.mybir as mybir
import concourse.tile as tile
from concourse.bass_utils import run_bass_kernel_spmd
from concourse.masks import make_identity

# ---- problem constants (hardcoded; kernel.py must be self-contained) ----
HID = 128
FEAT = 16
N_NODES = 100000
N_EDGES = 400000
N_PER = 100000
A_TOTAL = 300000
NUM_ACTIONS = 75000
N_CORES = 8

E_SH = N_EDGES // N_CORES   # 50000 edge rows per core
N_SH = N_NODES // N_CORES   # 12500 node rows per core
A_SH = N_PER // N_CORES     # 12500 action-effect rows per core

# Row->SBUF packing: C consecutive rows per partition, so a [128, C*W] tile
# covers 128*C rows with C*W*4B contiguous DMA descriptors.
C = 16              # edges/nodes DMA tiles [128, 2048]
T_E = 24            # 24*2048 = 49152 main edge rows
T_N = 6             # 6*2048 = 12288 main node rows
# edge remainder: 848 rows = [128,512](C=4) + [128,256](C=2) + [80,128]
# node remainder: 212 rows = [106, 256](C=2)

C_A = 8             # apack chunks [128, 384] cover 1024 rows (48 floats/row)
T_A = 12            # 12*1024 = 12288 main rows, tail 212 rows -> [106, 96]
A_TAIL = 212

QE_COLS = T_E * C + 7            # 391 = 384 main + 4 + 2 + 1
QN_COLS = (T_N * C + 2) * 3      # 294: 98 groups x 3 weights
PA_COLS = T_A * C_A * 3 + 6      # 294

# wts input [128, 388]: [0:3]=Wn columns (w3|wr|wc), [3:4]=u1 column,
# [4:388]=w48 (=[w2|w4|wv]) tiled x8 replicated across partitions
W_N = (0, 3)
W_U1 = (3, 4)
W_A48 = (4, 4 + C_A * 48)
WTS_COLS = 4 + C_A * 48

F32 = mybir.dt.float32
AX = mybir.AxisListType.X

_CACHE = {}


def _build_program():
    nc = bacc.Bacc("TRN2", target_bir_lowering=False, debug=False,
                   num_devices=N_CORES)

    edges_in = nc.dram_tensor("edges_in", [E_SH, HID], F32, kind="ExternalInput").ap()
    nodes_in = nc.dram_tensor("nodes_in", [N_SH, HID], F32, kind="ExternalInput").ap()
    apack_in = nc.dram_tensor("apack_in", [A_SH, 3 * FEAT], F32, kind="ExternalInput").ap()
    wts_in = nc.dram_tensor("wts_in", [128, WTS_COLS], F32, kind="ExternalInput").ap()

    qe_out = nc.dram_tensor("qe_out", [128, QE_COLS], F32, kind="ExternalOutput").ap()
    qn_out = nc.dram_tensor("qn_out", [128, QN_COLS], F32, kind="ExternalOutput").ap()
    pa_out = nc.dram_tensor("pa_out", [128, PA_COLS], F32, kind="ExternalOutput").ap()

    with tile.TileContext(nc) as tc:
        with (
            tc.tile_pool(name="wpool", bufs=1) as wpool,
            tc.tile_pool(name="dpool", bufs=6) as dpool,
            tc.tile_pool(name="adpool", bufs=3) as adpool,
            tc.tile_pool(name="dtpool", bufs=6) as dtpool,
            tc.tile_pool(name="atpool", bufs=2) as atpool,
            tc.tile_pool(name="accpool", bufs=1) as accpool,
            tc.tile_pool(name="pstr", bufs=5, space="PSUM") as pstr,
            tc.tile_pool(name="psacc", bufs=1, space="PSUM") as psacc,
        ):
            wt = wpool.tile([128, WTS_COLS], F32)
            nc.sync.dma_start(wt[:], wts_in[:])
            ident = wpool.tile([128, 128], F32)
            make_identity(nc, ident[:])
            wn_col = wt[:, W_N[0]:W_N[1]]
            u1_col = wt[:, W_U1[0]:W_U1[1]]
            a48b = wt[:, W_A48[0]:W_A48[1]]

            qe_ps = psacc.tile([128, QE_COLS], F32)
            qn_ps = psacc.tile([128, QN_COLS], F32)
            pa_acc = accpool.tile([128, PA_COLS], F32)

            e_main = edges_in[0:T_E * 128 * C, :].rearrange(
                "(t p c) f -> t p (c f)", p=128, c=C)
            e_m1 = edges_in[49152:49664, :].rearrange("(p c) f -> p (c f)", c=4)
            e_m2 = edges_in[49664:49920, :].rearrange("(p c) f -> p (c f)", c=2)
            e_tl = edges_in[49920:E_SH, :]
            n_main = nodes_in[0:T_N * 128 * C, :].rearrange(
                "(t p c) f -> t p (c f)", p=128, c=C)
            n_tl = nodes_in[T_N * 128 * C:N_SH, :].rearrange("(p c) f -> p (c f)", c=2)

            # tile specs: (src_ap, parts, n_groups, rhs_ap, acc_ps, [cols]).
            # Small remainder tiles go first: their DMA trigger latency hides
            # under the pipeline ramp instead of bubbling the steady stream.
            tiles = [
                (e_m1, 128, 4, u1_col, qe_ps, [(384 + g, 1) for g in range(4)]),
                (e_m2, 128, 2, u1_col, qe_ps, [(388 + g, 1) for g in range(2)]),
                (e_tl, 80, 1, u1_col, qe_ps, [(390, 1)]),
                (n_tl, 106, 2, wn_col, qn_ps,
                 [((T_N * C + g) * 3, 3) for g in range(2)]),
            ]
            tiles += [(e_main[t], 128, C, u1_col, qe_ps,
                       [(t * C + g, 1) for g in range(C)]) for t in range(T_E)]
            tiles += [(n_main[t], 128, C, wn_col, qn_ps,
                       [((t * C + g) * 3, 3) for g in range(C)]) for t in range(T_N - 1)]
            # last node tile split into 4 smaller tiles so the end-of-stream
            # pipeline drain (transpose->copy->dots->drain) is short
            n_last = nodes_in[(T_N - 1) * 128 * C:T_N * 128 * C, :].rearrange(
                "(t p c) f -> t p (c f)", p=128, c=4)
            tiles += [(n_last[q], 128, 4, wn_col, qn_ps,
                       [(((T_N - 1) * C + q * 4 + g) * 3, 3) for g in range(4)])
                      for q in range(4)]
            last_tile_of = {}
            for i, t in enumerate(tiles):
                last_tile_of[id(t[4])] = i
            drains = {id(qe_ps): (qe_ps, qe_out, QE_COLS),
                      id(qn_ps): (qn_ps, qn_out, QN_COLS)}

            # ---- action-feature chunks (DVE mul + 3D-view reduce) ----
            a_main = apack_in[0:T_A * 128 * C_A, :].rearrange(
                "(t p c) f -> t p (c f)", p=128, c=C_A)
            a_tl = apack_in[T_A * 128 * C_A:A_SH, :].rearrange(
                "(p c) f -> p (c f)", c=2)

            def emit_action_chunk(t):
                if t < T_A:
                    d = adpool.tile([128, C_A * 48], F32, tag="ad")
                    nc.sync.dma_start(d[:], a_main[t])
                    tmp = atpool.tile([128, C_A * 48], F32, tag="at")
                    nc.vector.tensor_mul(tmp[:], d[:], a48b)
                    nc.vector.reduce_sum(
                        pa_acc[:, t * C_A * 3:(t + 1) * C_A * 3],
                        tmp[:].rearrange("p (s f) -> p s f", f=FEAT), axis=AX)
                else:
                    ap_t = A_TAIL // 2  # 106
                    d = adpool.tile([128, 96], F32, tag="ad")
                    nc.sync.dma_start(d[:ap_t, :], a_tl)
                    tmp = atpool.tile([128, 96], F32, tag="at")
                    nc.vector.tensor_mul(tmp[:ap_t, :], d[:ap_t, :], a48b[:ap_t, :96])
                    nc.vector.reduce_sum(
                        pa_acc[:ap_t, T_A * C_A * 3:T_A * C_A * 3 + 6],
                        tmp[:ap_t, :].rearrange("p (s f) -> p s f", f=FEAT), axis=AX)
                if t == T_A:
                    nc.sync.dma_start(pa_out[:], pa_acc[:])

            # 2-slab software pipeline: emit dots two slabs behind the
            # transposes so PE never stalls on the PSUM->SBUF copy.
            pending = []
            state = {"slab": 0, "action": 0}

            def emit_dots():
                parts, gs, rhs, acc, cols, dT, last = pending.pop(0)
                for g in range(gs):
                    c0, ncol = cols[g]
                    nc.tensor.matmul(
                        acc[:parts, c0:c0 + ncol],
                        dT[:, g * 128:g * 128 + parts],
                        rhs[:, :])
                if last:
                    acc_ps, out_dram, cols_n = drains[id(acc)]
                    sb = accpool.tile([128, cols_n], F32, tag=f"sb{id(acc) % 97}")
                    if state["slab"] % 2 == 0:
                        nc.vector.tensor_copy(sb[:], acc_ps[:])
                    else:
                        nc.scalar.copy(sb[:], acc_ps[:])
                    nc.sync.dma_start(out_dram[:], sb[:])

            for ti, (src, parts, n_groups, rhs, acc, cols) in enumerate(tiles):
                d = dpool.tile([128, C * HID], F32, tag="d")
                nc.sync.dma_start(d[:parts, :n_groups * HID], src)
                for s in range(0, n_groups, 4):
                    gs = min(4, n_groups - s)
                    ps = pstr.tile([128, 512], F32, tag="ps")
                    for g in range(gs):
                        nc.tensor.transpose(
                            ps[:, g * 128:g * 128 + parts],
                            d[:parts, (s + g) * 128:(s + g + 1) * 128],
                            ident[:parts, :parts])
                    dT = dtpool.tile([128, 512], F32, tag="dT")
                    if state["slab"] % 2 == 0:
                        nc.vector.tensor_copy(dT[:, :gs * 128], ps[:, :gs * 128])
                    else:
                        nc.scalar.copy(dT[:, :gs * 128], ps[:, :gs * 128])
                    state["slab"] += 1
                    last = (ti == last_tile_of[id(acc)]) and s + 4 >= n_groups
                    pending.append((parts, gs, rhs, acc, cols[s:s + gs], dT, last))
                    if len(pending) > 2:
                        emit_dots()
                    if state["slab"] % 9 == 0 and state["action"] <= T_A:
                        emit_action_chunk(state["action"])
                        state["action"] += 1
            while pending:
                emit_dots()
            while state["action"] <= T_A:
                emit_action_chunk(state["action"])
                state["action"] += 1

    nc.compile()
    return nc


def _get_program():
    if "nc" not in _CACHE:
        _CACHE["nc"] = _build_program()
    return _CACHE["nc"]


def _unscramble_qe(arr):
    """[128, 391] -> [50000] in original row order."""
    main = arr[:, :T_E * C].reshape(128, T_E, C).transpose(1, 0, 2).reshape(-1)
    m1 = arr[:, 384:388].reshape(-1)                       # rows 49152 + p*4+g
    m2 = arr[:, 388:390].reshape(-1)                       # rows 49664 + p*2+g
    tail = arr[:80, 390]                                   # rows 49920 + p
    return np.concatenate([main, m1, m2, tail])


def _unscramble_qn(arr):
    """[128, 294] -> [12500, 3] (w3, wr, wc dots) in original row order."""
    tm = T_N - 1
    main = arr[:, :tm * C * 3].reshape(128, tm, C, 3).transpose(1, 0, 2, 3)
    main = main.reshape(-1, 3)                             # rows t*2048+p*16+g
    # last main tile was emitted as 4 C=4 sub-tiles: rows 10240+q*512+p*4+g
    split = arr[:, tm * C * 3:T_N * C * 3].reshape(128, 4, 4, 3)
    split = split.transpose(1, 0, 2, 3).reshape(-1, 3)
    tail = arr[:106, T_N * C * 3:].reshape(106, 2, 3).reshape(-1, 3)
    return np.concatenate([main, split, tail], axis=0)


def _unscramble_pa(arr):
    """[128, 294] -> [12500, 3] (ag.w2, an.w4, ae.wv) in original row order."""
    main = arr[:, :T_A * C_A * 3].reshape(128, T_A, C_A, 3).transpose(1, 0, 2, 3)
    main = main.reshape(-1, 3)                             # rows t*1024+p*8+j
    tail = arr[:A_TAIL // 2, T_A * C_A * 3:].reshape(106, 2, 3).reshape(-1, 3)
    return np.concatenate([main, tail], axis=0)


def kernel(**inputs):
    globs = inputs["globs"]
    nodes = np.ascontiguousarray(inputs["nodes"])
    edges = np.ascontiguousarray(inputs["edges"])
    action_globs = inputs["action_globs"]
    action_nodes = inputs["action_nodes"]
    action_edges = inputs["action_edges"]
    glob_W = inputs["glob_W"]; glob_b = inputs["glob_b"]
    node_W = inputs["node_W"]; node_b = inputs["node_b"]
    e1_W = inputs["e1_W"]; e1_b = inputs["e1_b"]
    e2_W = inputs["e2_W"]; e2_b = inputs["e2_b"]
    pol_W = inputs["pol_W"]; pol_b = inputs["pol_b"]
    row = inputs["row"]; col = inputs["col"]
    U = inputs["U"]; UA = inputs["UA"]; V = inputs["V"]; VA = inputs["VA"]
    E = inputs["E"]; EA = inputs["EA"]
    actions_batch = inputs["actions_batch"]

    # ---- fused weight vectors (float64 for accuracy; cast to f32 on device) ----
    polW = pol_W.astype(np.float64)[:, 0]                 # [128]
    g_f = glob_W.astype(np.float64) @ polW                # [144]
    n_f = node_W.astype(np.float64) @ polW                # [144]
    e2_f = e2_W.astype(np.float64) @ polW                 # [256]
    u1, u2 = e2_f[:HID], e2_f[HID:]
    e1_f = e1_W.astype(np.float64) @ u2                   # [272]
    w1, w2 = g_f[:HID], g_f[HID:]
    w3, w4 = n_f[:HID], n_f[HID:]
    wr, wv, wc = e1_f[:HID], e1_f[HID:HID + FEAT], e1_f[HID + FEAT:]
    cg = float(glob_b.astype(np.float64) @ polW)
    cn = float(node_b.astype(np.float64) @ polW)
    ce = float(e2_b.astype(np.float64) @ polW + e1_b.astype(np.float64) @ u2)

    wts = np.zeros((128, WTS_COLS), np.float32)
    wts[:, W_N[0]] = w3.astype(np.float32)
    wts[:, W_N[0] + 1] = wr.astype(np.float32)
    wts[:, W_N[0] + 2] = wc.astype(np.float32)
    wts[:, W_U1[0]] = u1.astype(np.float32)
    w48 = np.concatenate([w2, w4, wv]).astype(np.float32)
    wts[:, W_A48[0]:W_A48[1]] = np.tile(w48, (128, C_A))

    # packed action features [N_PER, 48] = [ag | an | ae]
    apack = np.empty((N_PER, 3 * FEAT), np.float32)
    apack[:, :FEAT] = action_globs
    apack[:, FEAT:2 * FEAT] = action_nodes
    apack[:, 2 * FEAT:] = action_edges

    nc = _get_program()
    in_maps = []
    for c in range(N_CORES):
        in_maps.append({
            "edges_in": edges[c * E_SH:(c + 1) * E_SH],
            "nodes_in": nodes[c * N_SH:(c + 1) * N_SH],
            "apack_in": apack[c * A_SH:(c + 1) * A_SH],
            "wts_in": wts,
        })
    res = run_bass_kernel_spmd(nc, in_maps, core_ids=list(range(N_CORES)))

    qe = np.empty(N_EDGES, np.float64)
    qn3 = np.empty((N_NODES, 3), np.float64)
    pa = np.empty((N_PER, 3), np.float64)
    for c in range(N_CORES):
        r = res.results[c]
        qe[c * E_SH:(c + 1) * E_SH] = _unscramble_qe(r["qe_out"])
        qn3[c * N_SH:(c + 1) * N_SH] = _unscramble_qn(r["qn_out"])
        pa[c * A_SH:(c + 1) * A_SH] = _unscramble_pa(r["pa_out"])
    qn, qr, qc = qn3[:, 0], qn3[:, 1], qn3[:, 2]

    # ---- host: gathers, scatter into action slots, segment sum ----
    qg = globs.astype(np.float64) @ w1                    # [512]
    p_g = qg[U] + pa[:, 0] + cg
    p_n = qn[V] + pa[:, 1] + cn
    p_e = qe[E] + qr[row[E]] + qc[col[E]] + pa[:, 2] + ce

    actions_p = np.zeros(A_TOTAL, np.float64)
    actions_p[UA] = p_g
    actions_p[VA] = p_n
    actions_p[EA] = p_e

    # torch-style _norm: consecutive group ids starting at actions_batch[0]
    ab = actions_batch.astype(np.int64)
    changed = ab[1:] != ab[:-1]
    seg = int(ab[0]) + np.concatenate([[0], np.cumsum(changed)])
    if seg[0] >= 0 and seg[-1] < NUM_ACTIONS:
        agg = np.bincount(seg, weights=actions_p, minlength=NUM_ACTIONS)[:NUM_ACTIONS]
    else:  # jax segment_sum drops out-of-range ids
        agg = np.zeros(NUM_ACTIONS, np.float64)
        valid = (seg >= 0) & (seg < NUM_ACTIONS)
        np.add.at(agg, seg[valid], actions_p[valid])

    out = agg + float(pol_b.astype(np.float64)[0])
    return out.astype(np.float32)[:, None]


# revision 13
# speedup vs baseline: 157762.3644x; 66976.2354x over previous
"""Trainium2 Bass kernel for nn_Actions_block_14388140442036 (gnn_message_passing).

The reference network is entirely linear (no activations), so the output
    out = segment_sum(actions) @ pol_W + pol_b
collapses to per-effect scalars:
    p[j] = actions[j] @ pol_W  (a dot product against fused weight vectors)
followed by a scalar segment-sum.  Folding pol_W through each branch:

  glob branch:  p_g[i] = (globs @ w1)[U[i]]     + action_globs[i]. w2 + cg
  node branch:  p_n[i] = (nodes @ w3)[V[i]]     + action_nodes[i]. w4 + cn
  edge branch:  p_e[i] = (edges @ u1)[E[i]] + (nodes @ wr)[row[E[i]]]
                        + (nodes @ wc)[col[E[i]]] + action_edges[i]. wv + ce

where  w1|w2 = glob_W @ pol_W,  w3|w4 = node_W @ pol_W,
       u1|u2 = e2_W @ pol_W,    wr|wv|wc = e1_W @ u2.

The memory-heavy work -- streaming edges (205MB), nodes (51MB, x3 weight
vectors) and packed action features (19MB) through per-row dot products --
runs on 8 NeuronCores (row-sharded, replicated weight columns).  Large C=16
row-packed DMA tiles (8KB descriptors, few dma_starts: the HWDGE trigger is
~625ns serialized per DMA; small remainder tiles are issued first so their
trigger latency hides under the pipeline ramp).  Per 128-row group the PE
transposes the tile (fp32 has no DMA transpose), DVE/ACT alternate copying
4-group PSUM slabs back to SBUF, and the PE then matmuls them against the
fused weight columns, accumulating dot-product columns directly in PSUM
banks.  The small action-feature matvecs run on the DVE (mul + 3D-view
reduce) in small chunks interleaved with the slab copies.  Each branch's
accumulator drains to HBM as soon as its last dots are emitted.  The host
does the tiny fused-weight precompute, the scalar gathers and the segment
sum.
"""

import numpy as np

import concourse.bacc as bacc
import concourse.mybir as mybir
import concourse.tile as tile
from concourse.bass_utils import run_bass_kernel_spmd
from concourse.masks import make_identity

# ---- problem constants (hardcoded; kernel.py must be self-contained) ----
HID = 128
FEAT = 16
N_NODES = 100000
N_EDGES = 400000
N_PER = 100000
A_TOTAL = 300000
NUM_ACTIONS = 75000
N_CORES = 8

E_SH = N_EDGES // N_CORES   # 50000 edge rows per core
N_SH = N_NODES // N_CORES   # 12500 node rows per core
A_SH = N_PER // N_CORES     # 12500 action-effect rows per core

# Row->SBUF packing: C consecutive rows per partition, so a [128, C*W] tile
# covers 128*C rows with C*W*4B contiguous DMA descriptors.
C = 16              # edges/nodes DMA tiles [128, 2048]
T_E = 24            # 24*2048 = 49152 main edge rows
T_N = 6             # 6*2048 = 12288 main node rows
# edge remainder: 848 rows = [128,512](C=4) + [128,256](C=2) + [80,128]
# node remainder: 212 rows = [106, 256](C=2)

C_A = 8             # apack chunks [128, 384] cover 1024 rows (48 floats/row)
T_A = 12            # 12*1024 = 12288 main rows, tail 212 rows -> [106, 96]
A_TAIL = 212

QE_COLS = T_E * C + 7            # 391 = 384 main + 4 + 2 + 1
QN_COLS = (T_N * C + 2) * 3      # 294: 98 groups x 3 weights
PA_COLS = T_A * C_A * 3 + 6      # 294

# wts input [128, 388]: [0:3]=Wn columns (w3|wr|wc), [3:4]=u1 column,
# [4:388]=w48 (=[w2|w4|wv]) tiled x8 replicated across partitions
W_N = (0, 3)
W_U1 = (3, 4)
W_A48 = (4, 4 + C_A * 48)
WTS_COLS = 4 + C_A * 48

F32 = mybir.dt.float32
AX = mybir.AxisListType.X

_CACHE = {}


def _build_program(repeat=1):
    nc = bacc.Bacc("TRN2", target_bir_lowering=False, debug=False,
                   num_devices=N_CORES)

    edges_in = nc.dram_tensor("edges_in", [E_SH, HID], F32, kind="ExternalInput").ap()
    nodes_in = nc.dram_tensor("nodes_in", [N_SH, HID], F32, kind="ExternalInput").ap()
    apack_in = nc.dram_tensor("apack_in", [A_SH, 3 * FEAT], F32, kind="ExternalInput").ap()
    wts_in = nc.dram_tensor("wts_in", [128, WTS_COLS], F32, kind="ExternalInput").ap()

    qe_out = nc.dram_tensor("qe_out", [128, QE_COLS], F32, kind="ExternalOutput").ap()
    qn_out = nc.dram_tensor("qn_out", [128, QN_COLS], F32, kind="ExternalOutput").ap()
    pa_out = nc.dram_tensor("pa_out", [128, PA_COLS], F32, kind="ExternalOutput").ap()

    with tile.TileContext(nc) as tc:
        with (
            tc.tile_pool(name="wpool", bufs=1) as wpool,
            tc.tile_pool(name="dpool", bufs=6) as dpool,
            tc.tile_pool(name="adpool", bufs=3) as adpool,
            tc.tile_pool(name="dtpool", bufs=6) as dtpool,
            tc.tile_pool(name="atpool", bufs=2) as atpool,
            tc.tile_pool(name="accpool", bufs=1) as accpool,
            tc.tile_pool(name="pstr", bufs=5, space="PSUM") as pstr,
            tc.tile_pool(name="psacc", bufs=1, space="PSUM") as psacc,
        ):
            wt = wpool.tile([128, WTS_COLS], F32)
            nc.sync.dma_start(wt[:], wts_in[:])
            ident = wpool.tile([128, 128], F32)
            make_identity(nc, ident[:])
            wn_col = wt[:, W_N[0]:W_N[1]]
            u1_col = wt[:, W_U1[0]:W_U1[1]]
            a48b = wt[:, W_A48[0]:W_A48[1]]

            qe_ps = psacc.tile([128, QE_COLS], F32)
            qn_ps = psacc.tile([128, QN_COLS], F32)
            pa_acc = accpool.tile([128, PA_COLS], F32)

            e_main = edges_in[0:T_E * 128 * C, :].rearrange(
                "(t p c) f -> t p (c f)", p=128, c=C)
            e_m1 = edges_in[49152:49664, :].rearrange("(p c) f -> p (c f)", c=4)
            e_m2 = edges_in[49664:49920, :].rearrange("(p c) f -> p (c f)", c=2)
            e_tl = edges_in[49920:E_SH, :]
            n_main = nodes_in[0:T_N * 128 * C, :].rearrange(
                "(t p c) f -> t p (c f)", p=128, c=C)
            n_tl = nodes_in[T_N * 128 * C:N_SH, :].rearrange("(p c) f -> p (c f)", c=2)

            # tile specs: (src_ap, parts, n_groups, rhs_ap, acc_ps, [cols]).
            # Small remainder tiles go first: their DMA trigger latency hides
            # under the pipeline ramp instead of bubbling the steady stream.
            tiles = [
                (e_m1, 128, 4, u1_col, qe_ps, [(384 + g, 1) for g in range(4)]),
                (e_m2, 128, 2, u1_col, qe_ps, [(388 + g, 1) for g in range(2)]),
                (e_tl, 80, 1, u1_col, qe_ps, [(390, 1)]),
                (n_tl, 106, 2, wn_col, qn_ps,
                 [((T_N * C + g) * 3, 3) for g in range(2)]),
            ]
            tiles += [(e_main[t], 128, C, u1_col, qe_ps,
                       [(t * C + g, 1) for g in range(C)]) for t in range(T_E)]
            tiles += [(n_main[t], 128, C, wn_col, qn_ps,
                       [((t * C + g) * 3, 3) for g in range(C)]) for t in range(T_N - 1)]
            # last node tile split into 4 smaller tiles so the end-of-stream
            # pipeline drain (transpose->copy->dots->drain) is short
            n_last = nodes_in[(T_N - 1) * 128 * C:T_N * 128 * C, :].rearrange(
                "(t p c) f -> t p (c f)", p=128, c=4)
            tiles += [(n_last[q], 128, 4, wn_col, qn_ps,
                       [(((T_N - 1) * C + q * 4 + g) * 3, 3) for g in range(4)])
                      for q in range(4)]
            last_tile_of = {}
            for i, t in enumerate(tiles):
                last_tile_of[id(t[4])] = i
            drains = {id(qe_ps): (qe_ps, qe_out, QE_COLS),
                      id(qn_ps): (qn_ps, qn_out, QN_COLS)}

            # ---- action-feature chunks (DVE mul + 3D-view reduce) ----
            a_main = apack_in[0:T_A * 128 * C_A, :].rearrange(
                "(t p c) f -> t p (c f)", p=128, c=C_A)
            a_tl = apack_in[T_A * 128 * C_A:A_SH, :].rearrange(
                "(p c) f -> p (c f)", c=2)

            def emit_action_chunk(t):
                if t < T_A:
                    d = adpool.tile([128, C_A * 48], F32, tag="ad")
                    nc.sync.dma_start(d[:], a_main[t])
                    tmp = atpool.tile([128, C_A * 48], F32, tag="at")
                    nc.vector.tensor_mul(tmp[:], d[:], a48b)
                    nc.vector.reduce_sum(
                        pa_acc[:, t * C_A * 3:(t + 1) * C_A * 3],
                        tmp[:].rearrange("p (s f) -> p s f", f=FEAT), axis=AX)
                else:
                    ap_t = A_TAIL // 2  # 106
                    d = adpool.tile([128, 96], F32, tag="ad")
                    nc.sync.dma_start(d[:ap_t, :], a_tl)
                    tmp = atpool.tile([128, 96], F32, tag="at")
                    nc.vector.tensor_mul(tmp[:ap_t, :], d[:ap_t, :], a48b[:ap_t, :96])
                    nc.vector.reduce_sum(
                        pa_acc[:ap_t, T_A * C_A * 3:T_A * C_A * 3 + 6],
                        tmp[:ap_t, :].rearrange("p (s f) -> p s f", f=FEAT), axis=AX)
                if t == T_A:
                    nc.sync.dma_start(pa_out[:], pa_acc[:])

            # 2-slab software pipeline: emit dots two slabs behind the
            # transposes so PE never stalls on the PSUM->SBUF copy.
            # (`repeat` re-emits the whole stream; used only for wall-clock
            # delta measurement of per-iteration HW time.)
            pending = []
            state = {"slab": 0, "action": 0}

            def emit_dots():
                parts, gs, rhs, acc, cols, dT, last = pending.pop(0)
                for g in range(gs):
                    c0, ncol = cols[g]
                    nc.tensor.matmul(
                        acc[:parts, c0:c0 + ncol],
                        dT[:, g * 128:g * 128 + parts],
                        rhs[:, :])
                if last:
                    acc_ps, out_dram, cols_n = drains[id(acc)]
                    sb = accpool.tile([128, cols_n], F32, tag=f"sb{id(acc) % 97}")
                    if state["slab"] % 2 == 0:
                        nc.vector.tensor_copy(sb[:], acc_ps[:])
                    else:
                        nc.scalar.copy(sb[:], acc_ps[:])
                    nc.sync.dma_start(out_dram[:], sb[:])

            for _rep in range(repeat):
                state["action"] = 0
                for ti, (src, parts, n_groups, rhs, acc, cols) in enumerate(tiles):
                    d = dpool.tile([128, C * HID], F32, tag="d")
                    nc.sync.dma_start(d[:parts, :n_groups * HID], src)
                    for s in range(0, n_groups, 4):
                        gs = min(4, n_groups - s)
                        ps = pstr.tile([128, 512], F32, tag="ps")
                        for g in range(gs):
                            nc.tensor.transpose(
                                ps[:, g * 128:g * 128 + parts],
                                d[:parts, (s + g) * 128:(s + g + 1) * 128],
                                ident[:parts, :parts])
                        dT = dtpool.tile([128, 512], F32, tag="dT")
                        if state["slab"] % 2 == 0:
                            nc.vector.tensor_copy(dT[:, :gs * 128], ps[:, :gs * 128])
                        else:
                            nc.scalar.copy(dT[:, :gs * 128], ps[:, :gs * 128])
                        state["slab"] += 1
                        last = (ti == last_tile_of[id(acc)]) and s + 4 >= n_groups
                        pending.append((parts, gs, rhs, acc, cols[s:s + gs], dT, last))
                        if len(pending) > 2:
                            emit_dots()
                        if state["slab"] % 9 == 0 and state["action"] <= T_A:
                            emit_action_chunk(state["action"])
                            state["action"] += 1
                while pending:
                    emit_dots()
                while state["action"] <= T_A:
                    emit_action_chunk(state["action"])
                    state["action"] += 1

    nc.compile()
    return nc


def _get_program():
    if "nc" not in _CACHE:
        _CACHE["nc"] = _build_program()
    return _CACHE["nc"]


def _unscramble_qe(arr):
    """[128, 391] -> [50000] in original row order."""
    main = arr[:, :T_E * C].reshape(128, T_E, C).transpose(1, 0, 2).reshape(-1)
    m1 = arr[:, 384:388].reshape(-1)                       # rows 49152 + p*4+g
    m2 = arr[:, 388:390].reshape(-1)                       # rows 49664 + p*2+g
    tail = arr[:80, 390]                                   # rows 49920 + p
    return np.concatenate([main, m1, m2, tail])


def _unscramble_qn(arr):
    """[128, 294] -> [12500, 3] (w3, wr, wc dots) in original row order."""
    tm = T_N - 1
    main = arr[:, :tm * C * 3].reshape(128, tm, C, 3).transpose(1, 0, 2, 3)
    main = main.reshape(-1, 3)                             # rows t*2048+p*16+g
    # last main tile was emitted as 4 C=4 sub-tiles: rows 10240+q*512+p*4+g
    split = arr[:, tm * C * 3:T_N * C * 3].reshape(128, 4, 4, 3)
    split = split.transpose(1, 0, 2, 3).reshape(-1, 3)
    tail = arr[:106, T_N * C * 3:].reshape(106, 2, 3).reshape(-1, 3)
    return np.concatenate([main, split, tail], axis=0)


def _unscramble_pa(arr):
    """[128, 294] -> [12500, 3] (ag.w2, an.w4, ae.wv) in original row order."""
    main = arr[:, :T_A * C_A * 3].reshape(128, T_A, C_A, 3).transpose(1, 0, 2, 3)
    main = main.reshape(-1, 3)                             # rows t*1024+p*8+j
    tail = arr[:A_TAIL // 2, T_A * C_A * 3:].reshape(106, 2, 3).reshape(-1, 3)
    return np.concatenate([main, tail], axis=0)


def kernel(**inputs):
    inputs = {k: np.asarray(v) for k, v in inputs.items()}
    globs = inputs["globs"]
    nodes = np.ascontiguousarray(inputs["nodes"])
    edges = np.ascontiguousarray(inputs["edges"])
    action_globs = inputs["action_globs"]
    action_nodes = inputs["action_nodes"]
    action_edges = inputs["action_edges"]
    glob_W = inputs["glob_W"]; glob_b = inputs["glob_b"]
    node_W = inputs["node_W"]; node_b = inputs["node_b"]
    e1_W = inputs["e1_W"]; e1_b = inputs["e1_b"]
    e2_W = inputs["e2_W"]; e2_b = inputs["e2_b"]
    pol_W = inputs["pol_W"]; pol_b = inputs["pol_b"]
    row = inputs["row"]; col = inputs["col"]
    U = inputs["U"]; UA = inputs["UA"]; V = inputs["V"]; VA = inputs["VA"]
    E = inputs["E"]; EA = inputs["EA"]
    actions_batch = inputs["actions_batch"]

    # ---- fused weight vectors (float64 for accuracy; cast to f32 on device) ----
    polW = pol_W.astype(np.float64)[:, 0]                 # [128]
    g_f = glob_W.astype(np.float64) @ polW                # [144]
    n_f = node_W.astype(np.float64) @ polW                # [144]
    e2_f = e2_W.astype(np.float64) @ polW                 # [256]
    u1, u2 = e2_f[:HID], e2_f[HID:]
    e1_f = e1_W.astype(np.float64) @ u2                   # [272]
    w1, w2 = g_f[:HID], g_f[HID:]
    w3, w4 = n_f[:HID], n_f[HID:]
    wr, wv, wc = e1_f[:HID], e1_f[HID:HID + FEAT], e1_f[HID + FEAT:]
    cg = float(glob_b.astype(np.float64) @ polW)
    cn = float(node_b.astype(np.float64) @ polW)
    ce = float(e2_b.astype(np.float64) @ polW + e1_b.astype(np.float64) @ u2)

    wts = np.zeros((128, WTS_COLS), np.float32)
    wts[:, W_N[0]] = w3.astype(np.float32)
    wts[:, W_N[0] + 1] = wr.astype(np.float32)
    wts[:, W_N[0] + 2] = wc.astype(np.float32)
    wts[:, W_U1[0]] = u1.astype(np.float32)
    w48 = np.concatenate([w2, w4, wv]).astype(np.float32)
    wts[:, W_A48[0]:W_A48[1]] = np.tile(w48, (128, C_A))

    # packed action features [N_PER, 48] = [ag | an | ae]
    apack = np.empty((N_PER, 3 * FEAT), np.float32)
    apack[:, :FEAT] = action_globs
    apack[:, FEAT:2 * FEAT] = action_nodes
    apack[:, 2 * FEAT:] = action_edges

    nc = _get_program()
    in_maps = []
    for c in range(N_CORES):
        in_maps.append({
            "edges_in": edges[c * E_SH:(c + 1) * E_SH],
            "nodes_in": nodes[c * N_SH:(c + 1) * N_SH],
            "apack_in": apack[c * A_SH:(c + 1) * A_SH],
            "wts_in": wts,
        })
    res = run_bass_kernel_spmd(nc, in_maps, core_ids=list(range(N_CORES)))

    qe = np.empty(N_EDGES, np.float64)
    qn3 = np.empty((N_NODES, 3), np.float64)
    pa = np.empty((N_PER, 3), np.float64)
    for c in range(N_CORES):
        r = res.results[c]
        qe[c * E_SH:(c + 1) * E_SH] = _unscramble_qe(r["qe_out"])
        qn3[c * N_SH:(c + 1) * N_SH] = _unscramble_qn(r["qn_out"])
        pa[c * A_SH:(c + 1) * A_SH] = _unscramble_pa(r["pa_out"])
    qn, qr, qc = qn3[:, 0], qn3[:, 1], qn3[:, 2]

    # ---- host: gathers, scatter into action slots, segment sum ----
    qg = globs.astype(np.float64) @ w1                    # [512]
    p_g = qg[U] + pa[:, 0] + cg
    p_n = qn[V] + pa[:, 1] + cn
    p_e = qe[E] + qr[row[E]] + qc[col[E]] + pa[:, 2] + ce

    actions_p = np.zeros(A_TOTAL, np.float64)
    actions_p[UA] = p_g
    actions_p[VA] = p_n
    actions_p[EA] = p_e

    # torch-style _norm: consecutive group ids starting at actions_batch[0]
    ab = actions_batch.astype(np.int64)
    changed = ab[1:] != ab[:-1]
    seg = int(ab[0]) + np.concatenate([[0], np.cumsum(changed)])
    if seg[0] >= 0 and seg[-1] < NUM_ACTIONS:
        agg = np.bincount(seg, weights=actions_p, minlength=NUM_ACTIONS)[:NUM_ACTIONS]
    else:  # jax segment_sum drops out-of-range ids
        agg = np.zeros(NUM_ACTIONS, np.float64)
        valid = (seg >= 0) & (seg < NUM_ACTIONS)
        np.add.at(agg, seg[valid], actions_p[valid])

    out = agg + float(pol_b.astype(np.float64)[0])
    return out.astype(np.float32)[:, None]


# revision 14
# speedup vs baseline: 321707.0117x; 2.0392x over previous
"""Trainium2 Bass kernel for nn_Actions_block_14388140442036 (gnn_message_passing).

The reference network is entirely linear (no activations), so the output
    out = segment_sum(actions) @ pol_W + pol_b
collapses to per-effect scalars:
    p[j] = actions[j] @ pol_W  (a dot product against fused weight vectors)
followed by a scalar segment-sum.  Folding pol_W through each branch:

  glob branch:  p_g[i] = (globs @ w1)[U[i]]     + action_globs[i]. w2 + cg
  node branch:  p_n[i] = (nodes @ w3)[V[i]]     + action_nodes[i]. w4 + cn
  edge branch:  p_e[i] = (edges[E[i]] . u1) + (nodes @ wr)[row[E[i]]]
                        + (nodes @ wc)[col[E[i]]] + action_edges[i]. wv + ce

where  w1|w2 = glob_W @ pol_W,  w3|w4 = node_W @ pol_W,
       u1|u2 = e2_W @ pol_W,    wr|wv|wc = e1_W @ u2.

Only ~25% of edge rows are ever referenced (E gathers 100k effects from 400k
edges), so the edge features are gathered on the host (per the sharding
hint: data-parallel over action effects with gathered features) and only the
12.8MB of gathered rows stream through the device.  The nodes table is
needed nearly in full by three different gathers, so it streams once with
three fused weight vectors.

Per core (~15.4MB): large C=16 row-packed DMA tiles (8KB descriptors — the
HWDGE trigger is ~625ns serialized per DMA; small remainder tiles issue
first so their trigger latency hides under the pipeline ramp).  Per 128-row
group the PE transposes the tile (fp32 has no DMA transpose), DVE/ACT
alternate copying 4-group PSUM slabs back to SBUF, and the PE then matmuls
them against the fused weight columns, accumulating dot-product columns
directly in PSUM banks.  The small action-feature matvecs run on the DVE
(mul + 3D-view reduce) in chunks interleaved with the slab copies.  Each
branch's accumulator drains to HBM as soon as its last dots are emitted.
The host does the tiny fused-weight precompute, the scalar gathers and the
segment sum.
"""

import numpy as np

import concourse.bacc as bacc
import concourse.mybir as mybir
import concourse.tile as tile
from concourse.bass_utils import run_bass_kernel_spmd
from concourse.masks import make_identity

# ---- problem constants (hardcoded; kernel.py must be self-contained) ----
HID = 128
FEAT = 16
N_NODES = 100000
N_EDGES = 400000
N_PER = 100000
A_TOTAL = 300000
NUM_ACTIONS = 75000
N_CORES = 8

N_SH = N_NODES // N_CORES   # 12500 node rows per core
A_SH = N_PER // N_CORES     # 12500 action-effect rows per core (all branches)

# Row->SBUF packing: C consecutive rows per partition, so a [128, C*W] tile
# covers 128*C rows with C*W*4B contiguous DMA descriptors.
C = 16              # main DMA tiles [128, 2048]
T_M = 6             # 6*2048 = 12288 main rows per 12500-row stream
M_TAIL = 212        # rows 12288..12500 as [106, 256] (C=2)

C_A = 8             # apack chunks [128, 384] cover 1024 rows (48 floats/row)
T_A = 12            # 12*1024 = 12288 main rows, tail 212 rows -> [106, 96]
A_TAIL = 212

QG_COLS = T_M * C + 2            # 98 groups x 1 col (gathered-edge dots)
QN_COLS = (T_M * C + 2) * 3      # 294: 98 groups x 3 weights
PA_COLS = T_A * C_A * 3 + 6      # 294

# wts input [128, 388]: [0:3]=Wn columns (w3|wr|wc), [3:4]=u1 column,
# [4:388]=w48 (=[w2|w4|wv]) tiled x8 replicated across partitions
W_N = (0, 3)
W_U1 = (3, 4)
W_A48 = (4, 4 + C_A * 48)
WTS_COLS = 4 + C_A * 48

F32 = mybir.dt.float32
AX = mybir.AxisListType.X

_CACHE = {}


def _build_program(repeat=1):
    nc = bacc.Bacc("TRN2", target_bir_lowering=False, debug=False,
                   num_devices=N_CORES)

    eg_in = nc.dram_tensor("eg_in", [A_SH, HID], F32, kind="ExternalInput").ap()
    nodes_in = nc.dram_tensor("nodes_in", [N_SH, HID], F32, kind="ExternalInput").ap()
    apack_in = nc.dram_tensor("apack_in", [A_SH, 3 * FEAT], F32, kind="ExternalInput").ap()
    wts_in = nc.dram_tensor("wts_in", [128, WTS_COLS], F32, kind="ExternalInput").ap()

    qg_out = nc.dram_tensor("qg_out", [128, QG_COLS], F32, kind="ExternalOutput").ap()
    qn_out = nc.dram_tensor("qn_out", [128, QN_COLS], F32, kind="ExternalOutput").ap()
    pa_out = nc.dram_tensor("pa_out", [128, PA_COLS], F32, kind="ExternalOutput").ap()

    with tile.TileContext(nc) as tc:
        with (
            tc.tile_pool(name="wpool", bufs=1) as wpool,
            tc.tile_pool(name="dpool", bufs=6) as dpool,
            tc.tile_pool(name="adpool", bufs=3) as adpool,
            tc.tile_pool(name="dtpool", bufs=6) as dtpool,
            tc.tile_pool(name="atpool", bufs=2) as atpool,
            tc.tile_pool(name="accpool", bufs=1) as accpool,
            tc.tile_pool(name="pstr", bufs=5, space="PSUM") as pstr,
            tc.tile_pool(name="psacc", bufs=1, space="PSUM") as psacc,
        ):
            wt = wpool.tile([128, WTS_COLS], F32)
            nc.sync.dma_start(wt[:], wts_in[:])
            ident = wpool.tile([128, 128], F32)
            make_identity(nc, ident[:])
            wn_col = wt[:, W_N[0]:W_N[1]]
            u1_col = wt[:, W_U1[0]:W_U1[1]]
            a48b = wt[:, W_A48[0]:W_A48[1]]

            qg_ps = psacc.tile([128, QG_COLS], F32)
            qn_ps = psacc.tile([128, QN_COLS], F32)
            pa_acc = accpool.tile([128, PA_COLS], F32)

            g_main = eg_in[0:T_M * 128 * C, :].rearrange(
                "(t p c) f -> t p (c f)", p=128, c=C)
            g_tl = eg_in[T_M * 128 * C:A_SH, :].rearrange("(p c) f -> p (c f)", c=2)
            n_main = nodes_in[0:T_M * 128 * C, :].rearrange(
                "(t p c) f -> t p (c f)", p=128, c=C)
            n_tl = nodes_in[T_M * 128 * C:N_SH, :].rearrange("(p c) f -> p (c f)", c=2)
            # last node tile split into 4 C=4 sub-tiles so the end-of-stream
            # pipeline drain (transpose->copy->dots->drain) is short
            n_last = nodes_in[(T_M - 1) * 128 * C:T_M * 128 * C, :].rearrange(
                "(t p c) f -> t p (c f)", p=128, c=4)

            # tile specs: (src_ap, parts, n_groups, rhs_ap, acc_ps, [cols]).
            # Small remainder tiles go first: their DMA trigger latency hides
            # under the pipeline ramp instead of bubbling the steady stream.
            tiles = [
                (g_tl, 106, 2, u1_col, qg_ps,
                 [(T_M * C + g, 1) for g in range(2)]),
                (n_tl, 106, 2, wn_col, qn_ps,
                 [((T_M * C + g) * 3, 3) for g in range(2)]),
            ]
            tiles += [(g_main[t], 128, C, u1_col, qg_ps,
                       [(t * C + g, 1) for g in range(C)]) for t in range(T_M)]
            tiles += [(n_main[t], 128, C, wn_col, qn_ps,
                       [((t * C + g) * 3, 3) for g in range(C)])
                      for t in range(T_M - 1)]
            tiles += [(n_last[q], 128, 4, wn_col, qn_ps,
                       [(((T_M - 1) * C + q * 4 + g) * 3, 3) for g in range(4)])
                      for q in range(4)]
            last_tile_of = {}
            for i, t in enumerate(tiles):
                last_tile_of[id(t[4])] = i
            drains = {id(qg_ps): (qg_ps, qg_out, QG_COLS),
                      id(qn_ps): (qn_ps, qn_out, QN_COLS)}

            # ---- action-feature chunks (DVE mul + 3D-view reduce) ----
            a_main = apack_in[0:T_A * 128 * C_A, :].rearrange(
                "(t p c) f -> t p (c f)", p=128, c=C_A)
            a_tl = apack_in[T_A * 128 * C_A:A_SH, :].rearrange(
                "(p c) f -> p (c f)", c=2)

            def emit_action_chunk(t):
                if t < T_A:
                    d = adpool.tile([128, C_A * 48], F32, tag="ad")
                    nc.sync.dma_start(d[:], a_main[t])
                    tmp = atpool.tile([128, C_A * 48], F32, tag="at")
                    nc.vector.tensor_mul(tmp[:], d[:], a48b)
                    nc.vector.reduce_sum(
                        pa_acc[:, t * C_A * 3:(t + 1) * C_A * 3],
                        tmp[:].rearrange("p (s f) -> p s f", f=FEAT), axis=AX)
                else:
                    ap_t = A_TAIL // 2  # 106
                    d = adpool.tile([128, 96], F32, tag="ad")
                    nc.sync.dma_start(d[:ap_t, :], a_tl)
                    tmp = atpool.tile([128, 96], F32, tag="at")
                    nc.vector.tensor_mul(tmp[:ap_t, :], d[:ap_t, :], a48b[:ap_t, :96])
                    nc.vector.reduce_sum(
                        pa_acc[:ap_t, T_A * C_A * 3:T_A * C_A * 3 + 6],
                        tmp[:ap_t, :].rearrange("p (s f) -> p s f", f=FEAT), axis=AX)
                if t == T_A:
                    nc.sync.dma_start(pa_out[:], pa_acc[:])

            # 2-slab software pipeline: emit dots two slabs behind the
            # transposes so PE never stalls on the PSUM->SBUF copy.
            # (`repeat` re-emits the whole stream; used only for wall-clock
            # delta measurement of per-iteration HW time.)
            pending = []
            state = {"slab": 0, "action": 0}

            def emit_dots():
                parts, gs, rhs, acc, cols, dT, last = pending.pop(0)
                for g in range(gs):
                    c0, ncol = cols[g]
                    nc.tensor.matmul(
                        acc[:parts, c0:c0 + ncol],
                        dT[:, g * 128:g * 128 + parts],
                        rhs[:, :])
                if last:
                    acc_ps, out_dram, cols_n = drains[id(acc)]
                    sb = accpool.tile([128, cols_n], F32, tag=f"sb{id(acc) % 97}")
                    if state["slab"] % 2 == 0:
                        nc.vector.tensor_copy(sb[:], acc_ps[:])
                    else:
                        nc.scalar.copy(sb[:], acc_ps[:])
                    nc.sync.dma_start(out_dram[:], sb[:])

            for _rep in range(repeat):
                state["action"] = 0
                for ti, (src, parts, n_groups, rhs, acc, cols) in enumerate(tiles):
                    d = dpool.tile([128, C * HID], F32, tag="d")
                    nc.sync.dma_start(d[:parts, :n_groups * HID], src)
                    for s in range(0, n_groups, 4):
                        gs = min(4, n_groups - s)
                        ps = pstr.tile([128, 512], F32, tag="ps")
                        for g in range(gs):
                            nc.tensor.transpose(
                                ps[:, g * 128:g * 128 + parts],
                                d[:parts, (s + g) * 128:(s + g + 1) * 128],
                                ident[:parts, :parts])
                        dT = dtpool.tile([128, 512], F32, tag="dT")
                        if state["slab"] % 2 == 0:
                            nc.vector.tensor_copy(dT[:, :gs * 128], ps[:, :gs * 128])
                        else:
                            nc.scalar.copy(dT[:, :gs * 128], ps[:, :gs * 128])
                        state["slab"] += 1
                        last = (ti == last_tile_of[id(acc)]) and s + 4 >= n_groups
                        pending.append((parts, gs, rhs, acc, cols[s:s + gs], dT, last))
                        if len(pending) > 2:
                            emit_dots()
                        if state["slab"] % 4 == 0 and state["action"] <= T_A:
                            emit_action_chunk(state["action"])
                            state["action"] += 1
                while pending:
                    emit_dots()
                while state["action"] <= T_A:
                    emit_action_chunk(state["action"])
                    state["action"] += 1

    nc.compile()
    return nc


def _get_program():
    if "nc" not in _CACHE:
        _CACHE["nc"] = _build_program()
    return _CACHE["nc"]


def _unscramble_q1(arr):
    """[128, 98] -> [12500] (gathered-edge dots) in original row order."""
    main = arr[:, :T_M * C].reshape(128, T_M, C).transpose(1, 0, 2).reshape(-1)
    tail = arr[:106, T_M * C:].reshape(-1)                 # rows 12288 + p*2+g
    return np.concatenate([main, tail])


def _unscramble_qn(arr):
    """[128, 294] -> [12500, 3] (w3, wr, wc dots) in original row order."""
    tm = T_M - 1
    main = arr[:, :tm * C * 3].reshape(128, tm, C, 3).transpose(1, 0, 2, 3)
    main = main.reshape(-1, 3)                             # rows t*2048+p*16+g
    # last main tile was emitted as 4 C=4 sub-tiles: rows 10240+q*512+p*4+g
    split = arr[:, tm * C * 3:T_M * C * 3].reshape(128, 4, 4, 3)
    split = split.transpose(1, 0, 2, 3).reshape(-1, 3)
    tail = arr[:106, T_M * C * 3:].reshape(106, 2, 3).reshape(-1, 3)
    return np.concatenate([main, split, tail], axis=0)


def _unscramble_pa(arr):
    """[128, 294] -> [12500, 3] (ag.w2, an.w4, ae.wv) in original row order."""
    main = arr[:, :T_A * C_A * 3].reshape(128, T_A, C_A, 3).transpose(1, 0, 2, 3)
    main = main.reshape(-1, 3)                             # rows t*1024+p*8+j
    tail = arr[:A_TAIL // 2, T_A * C_A * 3:].reshape(106, 2, 3).reshape(-1, 3)
    return np.concatenate([main, tail], axis=0)


def kernel(**inputs):
    inputs = {k: np.asarray(v) for k, v in inputs.items()}
    globs = inputs["globs"]
    nodes = np.ascontiguousarray(inputs["nodes"])
    edges = np.ascontiguousarray(inputs["edges"])
    action_globs = inputs["action_globs"]
    action_nodes = inputs["action_nodes"]
    action_edges = inputs["action_edges"]
    glob_W = inputs["glob_W"]; glob_b = inputs["glob_b"]
    node_W = inputs["node_W"]; node_b = inputs["node_b"]
    e1_W = inputs["e1_W"]; e1_b = inputs["e1_b"]
    e2_W = inputs["e2_W"]; e2_b = inputs["e2_b"]
    pol_W = inputs["pol_W"]; pol_b = inputs["pol_b"]
    row = inputs["row"]; col = inputs["col"]
    U = inputs["U"]; UA = inputs["UA"]; V = inputs["V"]; VA = inputs["VA"]
    E = inputs["E"]; EA = inputs["EA"]
    actions_batch = inputs["actions_batch"]

    # ---- fused weight vectors (float64 for accuracy; cast to f32 on device) ----
    polW = pol_W.astype(np.float64)[:, 0]                 # [128]
    g_f = glob_W.astype(np.float64) @ polW                # [144]
    n_f = node_W.astype(np.float64) @ polW                # [144]
    e2_f = e2_W.astype(np.float64) @ polW                 # [256]
    u1, u2 = e2_f[:HID], e2_f[HID:]
    e1_f = e1_W.astype(np.float64) @ u2                   # [272]
    w1, w2 = g_f[:HID], g_f[HID:]
    w3, w4 = n_f[:HID], n_f[HID:]
    wr, wv, wc = e1_f[:HID], e1_f[HID:HID + FEAT], e1_f[HID + FEAT:]
    cg = float(glob_b.astype(np.float64) @ polW)
    cn = float(node_b.astype(np.float64) @ polW)
    ce = float(e2_b.astype(np.float64) @ polW + e1_b.astype(np.float64) @ u2)

    wts = np.zeros((128, WTS_COLS), np.float32)
    wts[:, W_N[0]] = w3.astype(np.float32)
    wts[:, W_N[0] + 1] = wr.astype(np.float32)
    wts[:, W_N[0] + 2] = wc.astype(np.float32)
    wts[:, W_U1[0]] = u1.astype(np.float32)
    w48 = np.concatenate([w2, w4, wv]).astype(np.float32)
    wts[:, W_A48[0]:W_A48[1]] = np.tile(w48, (128, C_A))

    # gathered edge features for the edge branch (only ~25% of edge rows are
    # referenced; shipping the gathered rows quarters the edge stream)
    eg = edges[E]                                          # [N_PER, 128]

    # packed action features [N_PER, 48] = [ag | an | ae]
    apack = np.empty((N_PER, 3 * FEAT), np.float32)
    apack[:, :FEAT] = action_globs
    apack[:, FEAT:2 * FEAT] = action_nodes
    apack[:, 2 * FEAT:] = action_edges

    nc = _get_program()
    in_maps = []
    for c in range(N_CORES):
        in_maps.append({
            "eg_in": eg[c * A_SH:(c + 1) * A_SH],
            "nodes_in": nodes[c * N_SH:(c + 1) * N_SH],
            "apack_in": apack[c * A_SH:(c + 1) * A_SH],
            "wts_in": wts,
        })
    res = run_bass_kernel_spmd(nc, in_maps, core_ids=list(range(N_CORES)))

    qe_g = np.empty(N_PER, np.float64)                    # edges[E].u1, effect order
    qn3 = np.empty((N_NODES, 3), np.float64)
    pa = np.empty((N_PER, 3), np.float64)
    for c in range(N_CORES):
        r = res.results[c]
        qe_g[c * A_SH:(c + 1) * A_SH] = _unscramble_q1(r["qg_out"])
        qn3[c * N_SH:(c + 1) * N_SH] = _unscramble_qn(r["qn_out"])
        pa[c * A_SH:(c + 1) * A_SH] = _unscramble_pa(r["pa_out"])
    qn, qr, qc = qn3[:, 0], qn3[:, 1], qn3[:, 2]

    # ---- host: gathers, scatter into action slots, segment sum ----
    qg = globs.astype(np.float64) @ w1                    # [512]
    p_g = qg[U] + pa[:, 0] + cg
    p_n = qn[V] + pa[:, 1] + cn
    p_e = qe_g + qr[row[E]] + qc[col[E]] + pa[:, 2] + ce

    actions_p = np.zeros(A_TOTAL, np.float64)
    actions_p[UA] = p_g
    actions_p[VA] = p_n
    actions_p[EA] = p_e

    # torch-style _norm: consecutive group ids starting at actions_batch[0]
    ab = actions_batch.astype(np.int64)
    changed = ab[1:] != ab[:-1]
    seg = int(ab[0]) + np.concatenate([[0], np.cumsum(changed)])
    if seg[0] >= 0 and seg[-1] < NUM_ACTIONS:
        agg = np.bincount(seg, weights=actions_p, minlength=NUM_ACTIONS)[:NUM_ACTIONS]
    else:  # jax segment_sum drops out-of-range ids
        agg = np.zeros(NUM_ACTIONS, np.float64)
        valid = (seg >= 0) & (seg < NUM_ACTIONS)
        np.add.at(agg, seg[valid], actions_p[valid])

    out = agg + float(pol_b.astype(np.float64)[0])
    return out.astype(np.float32)[:, None]
